# revision 1
# baseline (speedup 1.0000x reference)
"""GAT kernel for Trainium2, SPMD over 8 NeuronCores.

Math: the reference GAT variant computes attention logits e[b,h,i,j] that do
NOT depend on j (the "untransposed Wh2" formulation), so softmax over a row
whose support (adj!=0) carries a constant value collapses to 1/deg(i) on the
support and 0 elsewhere (NEG_INF -> exp underflow -> exactly 0 in fp32).
Hence, per batch element b:

    out[b] = elu( diag(1/deg_b) @ (adj_b * adj_weight_b) @ (h_b @ W) )

with deg_b[i] = sum_j adj_b[i,j].  The result is head-independent and `a` is
unused.  Sharding: data-parallel over batch (B == n_cores == 8).

Device layouts are chosen host-side so the kernel needs no on-chip
transposes: h, adj, adj_weight are fed transposed (contraction index on the
SBUF partition dim), W is fed natural.  adj is fed as uint8 (values 0/1).

ELU identity used on device: elu(x) = min(exp(x) - 1, relu(x)), exact for
all x (including exp overflow -> inf, where min picks relu(x) = x).
"""

import os

import numpy as np

import concourse.bass as bass
import concourse.tile as tile
from concourse import bacc, mybir
from concourse.bass import ts
from concourse.bass_utils import run_bass_kernel_spmd

B, N, D = 8, 512, 1024
P = 128  # SBUF partitions
NB = N // P  # 4 row blocks
DB = D // P  # 8 contraction blocks
FH = D // 512  # 2 free-dim halves of 512

F32 = mybir.dt.float32
F32R = mybir.dt.float32r
U8 = mybir.dt.uint8
F16 = mybir.dt.float16
AF = mybir.ActivationFunctionType
ALU = mybir.AluOpType


def build_nc():
    nc = bacc.Bacc("TRN2", target_bir_lowering=False, debug=False, num_devices=B)

    hT = nc.dram_tensor("hT", [D, N], F16, kind="ExternalInput").ap()
    W = nc.dram_tensor("W", [D, D], F16, kind="ExternalInput").ap()
    adjT = nc.dram_tensor("adjT", [N, N], U8, kind="ExternalInput").ap()
    adjwT = nc.dram_tensor("adjwT", [N, N], F16, kind="ExternalInput").ap()
    out = nc.dram_tensor("out", [N, D], F32, kind="ExternalOutput").ap()
    out_r = out.rearrange("(n p) f -> p n f", p=P)
    hT_r = hT.rearrange("(n p) i -> p n i", p=P)    # [128, 8, 512]
    W_r = W.rearrange("(n p) f -> p n f", p=P)      # [128, 8, 1024]
    adjT_r = adjT.rearrange("(n p) i -> p n i", p=P)
    adjwT_r = adjwT.rearrange("(n p) i -> p n i", p=P)

    with tile.TileContext(nc) as tc:
        with (
            tc.tile_pool(name="singles", bufs=1) as singles,
            tc.tile_pool(name="work", bufs=4) as work,
            tc.tile_pool(name="outp", bufs=4) as outp,
            tc.tile_pool(name="psum", bufs=8, space="PSUM") as psum,
        ):
            # ---- resident SBUF tensors --------------------------------
            # h/W in d-pair tiles so one DMA covers two contraction chunks
            hT_d = [singles.tile([P, 2, N], F16, name=f"hT{d}", tag=f"hT{d}") for d in range(DB // 2)]
            W_d = [singles.tile([P, 2, D], F16, name=f"W{d}", tag=f"W{d}") for d in range(DB // 2)]
            adjT_sb = singles.tile([P, NB, N], U8)
            adjwT_sb = singles.tile([P, NB, N], F16)
            S = singles.tile([P, N], F16)  # sum_j adj^T (partial deg)
            MT_sb = singles.tile([P, NB, N], F16)  # (adj * adj_weight)^T
            Wh_sb = singles.tile([P, NB, D], F16)  # [p, j_blk, f]
            ones = singles.tile([P, 1], F16)
            junk = singles.tile([P, 640], F16)
            r_sb = singles.tile([P, NB], F32)  # 1/deg, column layout

            # ---- input DMAs, ordered to match PE consumption ----------
            # first two chunks arrive as small DMAs so MM1 starts early;
            # the rest ride full-size pair DMAs.
            for m in range(2):
                nc.sync.dma_start(hT_d[0][:, m], hT_r[:, m])
                nc.sync.dma_start(W_d[0][:, m], W_r[:, m])
            for d in range(1, DB // 2):
                nc.sync.dma_start(hT_d[d], hT_r[:, ts(d, 2)])
                nc.sync.dma_start(W_d[d], W_r[:, ts(d, 2)])
            nc.sync.dma_start(adjwT_sb, adjwT_r)
            nc.sync.dma_start(adjT_sb, adjT_r)

            nc.vector.memset(junk, 0.0)
            nc.vector.memset(ones, 1.0)

            # ---- PE warmup: dummy matmuls on zeros while DMA streams --
            # keeps the PE HAM activity window busy so real matmuls run
            # at 2.4 GHz from the start.
            warm_ps = psum.tile([P, 512], F32, tag="mm")
            for _ in range(6):
                nc.tensor.matmul(
                    warm_ps, junk[:, :P], junk[:, P:640], start=True, stop=True
                )

            # ---- DVE: deg pre-sum directly from u8 adjT ---------------
            t01 = work.tile([P, N], F16, tag="s01")
            nc.vector.tensor_add(t01, adjT_sb[:, 0], adjT_sb[:, 1])
            nc.vector.tensor_add(S, adjT_sb[:, 2], adjT_sb[:, 3])
            nc.vector.tensor_add(S, t01, S)

            # ---- DVE: M^T = adjT * adjwT (u8 x f32 -> f32r) -----------
            for j in range(NB):
                nc.vector.tensor_mul(MT_sb[:, j], adjT_sb[:, j], adjwT_sb[:, j])

            # ---- PE MM1: Wh = h @ W, d-outer so chunks stream ---------
            ps1 = [psum.tile([P, 512], F32, name=f"ps1_{k}", tag="mm") for k in range(NB * FH)]
            for d in range(DB):
                for i in range(NB):
                    for f in range(FH):
                        nc.tensor.matmul(
                            ps1[i * FH + f],
                            hT_d[d // 2][:, d % 2, ts(i, P)],
                            W_d[d // 2][:, d % 2, ts(f, 512)],
                            start=(d == 0),
                            stop=(d == DB - 1),
                        )
            # deg matmuls (fp16) fill the PE gap while Wh evacuates.
            deg_ps = psum.tile([P, NB], F32, tag="mm")
            for i in range(NB):
                nc.tensor.matmul(
                    deg_ps[:, i : i + 1], S[:, ts(i, P)], ones, start=True, stop=True
                )
            nc.vector.reciprocal(r_sb, deg_ps)

            for f in range(FH):
                for i in range(NB):
                    dst = Wh_sb[:, i, ts(f, 512)]
                    if i % 2 == 0:
                        nc.scalar.copy(dst, ps1[i * FH + f])
                    else:
                        nc.vector.tensor_copy(dst, ps1[i * FH + f])

            # ---- PE MM2 + fused scale + ELU ---------------------------
            # x = r[i] * psum;  elu(x) = min(exp(x) - 1, relu(x))
            for f in range(FH):
                for i in range(NB):
                    ps2 = psum.tile([P, 512], F32, tag="mm")
                    for j in range(NB):
                        nc.tensor.matmul(
                            ps2,
                            MT_sb[:, j, ts(i, P)],
                            Wh_sb[:, j, ts(f, 512)],
                            start=(j == 0),
                            stop=(j == NB - 1),
                        )
                    r_i = r_sb[:, i : i + 1]
                    exp_t = work.tile([P, 512], F32, tag="exp")
                    nc.scalar.activation(exp_t, ps2, AF.Exp, scale=r_i)
                    relu_t = work.tile([P, 512], F32, tag="relu")
                    if f * NB + i >= 5:
                        nc.scalar.activation(relu_t, ps2, AF.Relu, scale=r_i)
                    else:
                        nc.vector.tensor_scalar(
                            relu_t, ps2, r_i, 0.0, op0=ALU.mult, op1=ALU.max
                        )
                    o_t = outp.tile([P, 512], F32)
                    nc.vector.scalar_tensor_tensor(
                        o_t, exp_t, -1.0, relu_t, op0=ALU.add, op1=ALU.min
                    )
                    nc.gpsimd.dma_start(out_r[:, i, ts(f, 512)], o_t)

    nc.compile()
    return nc


_NC = None


def _get_nc():
    global _NC
    if _NC is None:
        _NC = build_nc()
    return _NC


def _in_maps(h, adj, adj_weight, W):
    h = np.ascontiguousarray(np.asarray(h, dtype=np.float32))
    adj = np.asarray(adj)
    adj_weight = np.ascontiguousarray(np.asarray(adj_weight, dtype=np.float32))
    Wf = np.ascontiguousarray(np.asarray(W, dtype=np.float32).reshape(D, D).astype(np.float16))
    hT = np.ascontiguousarray(h.transpose(0, 2, 1).astype(np.float16))
    adjT = np.ascontiguousarray(adj.transpose(0, 2, 1).astype(np.uint8))
    adjwT = np.ascontiguousarray(adj_weight.transpose(0, 2, 1).astype(np.float16))
    return [
        {"hT": hT[b], "W": Wf, "adjT": adjT[b], "adjwT": adjwT[b]} for b in range(B)
    ]


def _run(h, adj, adj_weight, W, a=None, trace=False, **trace_kw):
    nc = _get_nc()
    res = run_bass_kernel_spmd(
        nc, _in_maps(h, adj, adj_weight, W), core_ids=list(range(B)),
        trace=trace, **trace_kw,
    )
    out = np.stack([res.results[c]["out"] for c in range(B)], axis=0)
    return out.astype(np.float32), res


def kernel(h, adj, adj_weight, W, a=None, **_ignored):
    # The NTFF trace path needs an axon hook module this container lacks;
    # make sure an ambient BASS_TRACE can't divert the graded run into it.
    os.environ["BASS_NEVER_TRACE"] = "1"
    out, _ = _run(h, adj, adj_weight, W)
    return out



# revision 2
# speedup vs baseline: 1.1255x; 1.1255x over previous
"""GAT kernel for Trainium2, SPMD over 8 NeuronCores (v2).

Math: the reference GAT variant computes attention logits e[b,h,i,j] that do
NOT depend on j (the "untransposed Wh2" formulation), so softmax over a row
whose support (adj!=0) carries a constant value collapses to 1/deg(i) on the
support and 0 elsewhere (NEG_INF -> exp underflow -> exactly 0 in fp32).
Hence, per batch element b:

    out[b] = elu( diag(1/deg_b) @ (adj_b * adj_weight_b) @ (h_b @ W) )

with deg_b[i] = sum_j adj_b[i,j].  The result is head-independent and `a` is
unused.  Sharding: data-parallel over batch (B == n_cores == 8).

v2 schedule (per core), designed so the PE never idles after warm-up:
  PE   : warmup -> MM1-f0 (d-outer, streams hw chunks) -> MM1-f1 (+deg MMs)
         -> MM2-f0 -> MM2-f1
  DMA  : 12 big contiguous input DMAs on sync-HWDGE in exact consumption
         order (hw d-chunks carry hT_d and W_f0_d in one transfer); fp16
         output tiles stream back on the same queue as they are produced.
  ACT  : Wh PSUM->SBUF copies (half), exp leg of ELU, relu leg (odd tiles)
  DVE  : deg pre-sums, M^T=adj*w, Wh copies (half), relu leg (even), min
ELU identity used on device: elu(y) = min(exp(y) - 1, relu(y)), exact.
"""

import os

import numpy as np

import concourse.bass as bass
import concourse.tile as tile
from concourse import bacc, mybir
from concourse.bass import ts
from concourse.bass_utils import run_bass_kernel_spmd

B, N, D = 8, 512, 1024
P = 128  # SBUF partitions
NB = N // P  # 4 row blocks
DB = D // P  # 8 contraction blocks

F32 = mybir.dt.float32
U8 = mybir.dt.uint8
F16 = mybir.dt.float16
AF = mybir.ActivationFunctionType
ALU = mybir.AluOpType


def build_nc():
    nc = bacc.Bacc("TRN2", target_bir_lowering=False, debug=False, num_devices=B)

    # DRAM inputs, host-packed so every DMA is one dense contiguous block
    # with 1-2KB per-partition lines.
    hw = nc.dram_tensor("hw", [DB, P, 1024], F16, kind="ExternalInput").ap()
    wf1 = nc.dram_tensor("wf1", [2, P, 4, 512], F16, kind="ExternalInput").ap()
    adjT = nc.dram_tensor("adjT", [P, NB, N], U8, kind="ExternalInput").ap()
    adjwT = nc.dram_tensor("adjwT", [P, NB, N], F16, kind="ExternalInput").ap()
    out = nc.dram_tensor("out", [2, NB, P, 512], F16, kind="ExternalOutput").ap()

    with tile.TileContext(nc) as tc:
        with (
            tc.tile_pool(name="singles", bufs=1) as singles,
            tc.tile_pool(name="work", bufs=4) as work,
            tc.tile_pool(name="outp", bufs=4) as outp,
            tc.tile_pool(name="psum", bufs=8, space="PSUM") as psum,
        ):
            # ---- resident SBUF tensors --------------------------------
            hw_sb = [singles.tile([P, 1024], F16, name=f"hw{d}", tag=f"hw{d}") for d in range(DB)]
            wf1_sb = [singles.tile([P, 4, 512], F16, name=f"wf1_{k}", tag=f"wf1_{k}") for k in range(2)]
            adjT_sb = singles.tile([P, NB, N], U8)
            adjwT_sb = singles.tile([P, NB, N], F16)
            MT_sb = singles.tile([P, NB, N], F16)  # (adj * adj_weight)^T
            Wh_sb = singles.tile([P, NB, 1024], F16)  # [jp, jblk, f]
            S = singles.tile([P, N], F16)  # partial deg over j-blocks
            t01 = singles.tile([P, N], F16)
            ones = singles.tile([P, 1], F16)
            junk = singles.tile([P, 256], F16)
            r_sb = singles.tile([P, NB], F32)  # 1/deg per i-block column

            # ---- input DMAs on sync HWDGE, in consumption order -------
            for d in range(DB):
                nc.sync.dma_start(hw_sb[d], hw[d])
            nc.sync.dma_start(adjT_sb, adjT)
            for k in range(2):
                nc.sync.dma_start(wf1_sb[k], wf1[k])
            nc.sync.dma_start(adjwT_sb, adjwT)

            # ---- constants off the critical engines -------------------
            nc.gpsimd.memset(junk, 0.0)
            nc.gpsimd.memset(ones, 1.0)

            # ---- PE warmup on zeros: keeps HAM busy while hw0 lands ---
            warm_ps = psum.tile([P, 512], F32, tag="mm")
            for _ in range(8):
                nc.tensor.matmul(
                    warm_ps[:, :P], junk[:, :P], junk[:, P:256], start=True, stop=True
                )

            # ---- MM1 f0-half: Wh[:, :512] = h @ W[:, :512], d streams -
            ps_f0 = [psum.tile([P, 512], F32, name=f"psf0_{i}", tag="mm") for i in range(NB)]
            for d in range(DB):
                for i in range(NB):
                    nc.tensor.matmul(
                        ps_f0[i],
                        hw_sb[d][:, ts(i, P)],
                        hw_sb[d][:, 512:1024],
                        start=(d == 0),
                        stop=(d == DB - 1),
                    )

            # ---- DVE: deg pre-sum from u8 adjT (ready before deg MMs) -
            nc.vector.tensor_add(t01, adjT_sb[:, 0], adjT_sb[:, 1])
            nc.vector.tensor_add(S, adjT_sb[:, 2], adjT_sb[:, 3])
            nc.vector.tensor_add(S, t01, S)

            # ---- Wh f0 evacuation, split ACT/DVE ----------------------
            for i in range(NB):
                dst = Wh_sb[:, i, 0:512]
                if i % 2 == 0:
                    nc.scalar.copy(dst, ps_f0[i])
                else:
                    nc.vector.tensor_copy(dst, ps_f0[i])

            # ---- MM1 f1-half d=0 pass, then deg matmuls, then rest ----
            ps_f1 = [psum.tile([P, 512], F32, name=f"psf1_{i}", tag="mm") for i in range(NB)]
            for i in range(NB):
                nc.tensor.matmul(
                    ps_f1[i], hw_sb[0][:, ts(i, P)], wf1_sb[0][:, 0],
                    start=True, stop=False,
                )
            deg_ps = psum.tile([P, NB], F32, tag="mm")
            for i in range(NB):
                nc.tensor.matmul(
                    deg_ps[:, i : i + 1], S[:, ts(i, P)], ones, start=True, stop=True
                )
            for d in range(1, DB):
                for i in range(NB):
                    nc.tensor.matmul(
                        ps_f1[i],
                        hw_sb[d][:, ts(i, P)],
                        wf1_sb[d // 4][:, d % 4],
                        start=False,
                        stop=(d == DB - 1),
                    )

            nc.vector.reciprocal(r_sb, deg_ps)

            # ---- DVE: M^T = adjT * adjwT ------------------------------
            for j in range(NB):
                nc.vector.tensor_mul(MT_sb[:, j], adjT_sb[:, j], adjwT_sb[:, j])

            # ---- Wh f1 evacuation -------------------------------------
            for i in range(NB):
                dst = Wh_sb[:, i, 512:1024]
                if i % 2 == 0:
                    nc.scalar.copy(dst, ps_f1[i])
                else:
                    nc.vector.tensor_copy(dst, ps_f1[i])

            # ---- MM2 + fused 1/deg scale + ELU + output stream --------
            # y = r[i] * psum;  elu(y) = min(exp(y) - 1, relu(y))
            k = 0
            for f in range(2):
                for i in range(NB):
                    ps2 = psum.tile([P, 512], F32, tag="mm")
                    for j in range(NB):
                        nc.tensor.matmul(
                            ps2,
                            MT_sb[:, j, ts(i, P)],
                            Wh_sb[:, j, ts(f, 512)],
                            start=(j == 0),
                            stop=(j == NB - 1),
                        )
                    r_i = r_sb[:, i : i + 1]
                    exp_t = work.tile([P, 512], F16, tag="exp")
                    nc.scalar.activation(exp_t, ps2, AF.Exp, scale=r_i)
                    relu_t = work.tile([P, 512], F16, tag="relu")
                    if k % 2 == 1:
                        nc.scalar.activation(relu_t, ps2, AF.Relu, scale=r_i)
                    else:
                        nc.vector.tensor_scalar(
                            relu_t, ps2, r_i, 0.0, op0=ALU.mult, op1=ALU.max
                        )
                    o_t = outp.tile([P, 512], F16)
                    nc.vector.scalar_tensor_tensor(
                        o_t, exp_t, -1.0, relu_t, op0=ALU.add, op1=ALU.min
                    )
                    nc.sync.dma_start(out[f, i], o_t)
                    k += 1

    nc.compile()
    return nc


_NC = None


def _get_nc():
    global _NC
    if _NC is None:
        _NC = build_nc()
    return _NC


def _in_maps(h, adj, adj_weight, W):
    h = np.asarray(h, dtype=np.float32)
    adj = np.asarray(adj)
    adjw = np.asarray(adj_weight, dtype=np.float32)
    Wf = np.asarray(W, dtype=np.float32).reshape(D, D).astype(np.float16)

    # W columns split: f0 half rides with h chunks, f1 half separately.
    Wf0 = Wf[:, :512].reshape(DB, P, 512)  # [d, p, c]
    wf1 = np.ascontiguousarray(
        Wf[:, 512:].reshape(2, 4, P, 512).transpose(0, 2, 1, 3)
    )  # [k, p, m, c], d = 4k + m

    maps = []
    for b in range(B):
        hT3 = h[b].T.astype(np.float16).reshape(DB, P, N)  # [d, p, i]
        hw = np.ascontiguousarray(np.concatenate([hT3, Wf0], axis=2))  # [d,p,1024]
        adjTp = np.ascontiguousarray(
            adj[b].T.astype(np.uint8).reshape(NB, P, N).transpose(1, 0, 2)
        )  # [p, jb, i]
        adjwTp = np.ascontiguousarray(
            adjw[b].T.astype(np.float16).reshape(NB, P, N).transpose(1, 0, 2)
        )
        maps.append({"hw": hw, "wf1": wf1, "adjT": adjTp, "adjwT": adjwTp})
    return maps


def _unpack_out(res_out):
    # res_out: [2, NB, P, 512] f16 with [f, i, p, c] = O[128*i + p, 512*f + c]
    return (
        np.asarray(res_out)
        .transpose(1, 2, 0, 3)
        .reshape(N, D)
        .astype(np.float32)
    )


def _run(h, adj, adj_weight, W, a=None, trace=False, **trace_kw):
    nc = _get_nc()
    res = run_bass_kernel_spmd(
        nc, _in_maps(h, adj, adj_weight, W), core_ids=list(range(B)),
        trace=trace, **trace_kw,
    )
    out = np.stack([_unpack_out(res.results[c]["out"]) for c in range(B)], axis=0)
    return out, res


def kernel(h, adj, adj_weight, W, a=None, **_ignored):
    # The NTFF trace path needs an axon hook module this container lacks;
    # make sure an ambient BASS_TRACE can't divert the graded run into it.
    os.environ["BASS_NEVER_TRACE"] = "1"
    out, _ = _run(h, adj, adj_weight, W)
    return out
